# revision 1
# baseline (speedup 1.0000x reference)
"""Dinov2 SDPA self-attention on one TRN2 chip (8 NeuronCores).

Problem: hidden_states [4, 1370, 1024], 16 heads x 64 dim, fp32.

Sharding (hybrid data/tensor parallel): core c handles batch b = c//2 and
head-group g = c%2 (8 heads = 512 hidden columns). Each core computes its
Q/K/V projections from its batch's activations and runs attention for its
8 heads; the host concatenates the per-core [1370, 512] context outputs.
No on-chip collectives needed.

Per-core algorithm (all layouts transposed so softmax reductions become
matmul contractions):
  Xt = X^T in SBUF  [1024, 1370]
  Qt = Wq_g @ Xt + bq (per-partition bias)   [512, 1370]
  Kt = Wk_g @ Xt  (K bias is softmax-invariant -> dropped exactly)
  V  = X @ Wv_g^T + bv (natural layout, bias via DVE add; softmax weights
       sum to 1 so including bv here is exact)
  per head h: ST = Kt_h^T-tiles @ Qt_h = scores^T  [S, L] (contraction d=64,
       two heads packed in the PE array via row groups 0-63/64-127)
  P^T = exp(ST/8) (ACT, fused 1/sqrt(d) scale; no max-subtraction -- scores
       are bounded ~|4|, exp is safe)
  ctxT_ext = [V_h | 1]^T-style stationary @ P^T = [ctx^T; rowsums]  [65, L]
  PE-transpose 128-col slices -> [L_tile, 65], normalize by column 64 via
  DVE reciprocal + per-partition tensor_scalar multiply -> out staging.

Matmul operands are bf16 (fp32 PSUM accumulation); the unnormalized ctx
eviction stays fp32 (bf16 there would round ctx and rowsums independently
-- dominant error term). Validated vs fp32 reference: ~3.5e-3 of absmax.
"""

import os

import numpy as np
import ml_dtypes

import concourse.mybir as mybir
import concourse.tile as tile
from concourse import bacc
from concourse import bass_utils
from concourse.masks import make_identity

F32 = mybir.dt.float32
DT = mybir.dt.bfloat16
NPDT = ml_dtypes.bfloat16
AF = mybir.ActivationFunctionType

B = 4
L = 1370
HID = 1024
NH = 8            # heads per core
D = 64
QD = NH * D       # 512 projected dims per core
HP = NH // 2      # head pairs (PE row-group packing)
KC = HID // 128   # contraction chunks for projections

L_CHUNKS = [(0, 512), (512, 512), (1024, 346)]                      # moving/free dim
TILES = [(i * 128, min(128, L - i * 128)) for i in range((L + 127) // 128)]
NS = len(TILES)   # 11 (last tile 90)


def _body(nc, tc, xt_d, wq_d, wk_d, wv_d, bq_d, bv_d, out_d):
    with tc.tile_pool(name="persist", bufs=1) as pp:
        xt = pp.tile([128, KC, L], DT)
        wq = pp.tile([128, KC, QD], DT)
        wk = pp.tile([128, KC, QD], DT)
        wv = pp.tile([128, KC, QD], DT)
        qt = pp.tile([128, HP, L], DT)
        kt = pp.tile([128, HP, L], DT)
        vv = pp.tile([128, NS, NH, D + 1], DT)   # V tiles + ones column
        ost = pp.tile([128, NS, QD], F32)        # output staging, natural layout
        bqc = pp.tile([128, HP], F32)
        bvb = pp.tile([128, QD], F32)
        ident = pp.tile([128, 128], F32)

        make_identity(nc, ident[:, :])
        nc.vector.memset(vv[:, :, :, D:D + 1], 1.0)

        for k in range(KC):
            r = slice(k * 128, (k + 1) * 128)
            nc.sync.dma_start(xt[:, k, :], xt_d[r, :])
            nc.sync.dma_start(wq[:, k, :], wq_d[r, :])
            nc.sync.dma_start(wk[:, k, :], wk_d[r, :])
            nc.sync.dma_start(wv[:, k, :], wv_d[r, :])
        for hp in range(HP):
            nc.sync.dma_start(bqc[:, hp:hp + 1], bq_d[hp * 128:(hp + 1) * 128, :])
        nc.sync.dma_start(bvb[:, :], bv_d[:, :])

        # ---- projections ----
        with tc.tile_pool(name="pps", bufs=2, space="PSUM") as pps:
            for si, (s0, ss) in enumerate(TILES):
                vps = pps.tile([128, QD], F32, name="vps", tag="vps")
                for k in range(KC):
                    nc.tensor.matmul(vps[:ss, :], xt[:, k, s0:s0 + ss], wv[:, k, :],
                                     start=(k == 0), stop=(k == KC - 1))
                nc.vector.tensor_add(
                    vv[:ss, si, :, 0:D],
                    vps[:ss, :].rearrange("p (h d) -> p h d", h=NH),
                    bvb[:ss, :].rearrange("p (h d) -> p h d", h=NH),
                )

            for hp in range(HP):
                m = slice(hp * 128, (hp + 1) * 128)
                for (l0, ln) in L_CHUNKS:
                    qps = pps.tile([128, 512], F32, name="qps", tag="qps")
                    for k in range(KC):
                        nc.tensor.matmul(qps[:, :ln], wq[:, k, m], xt[:, k, l0:l0 + ln],
                                         start=(k == 0), stop=(k == KC - 1))
                    nc.scalar.activation(qt[:, hp, l0:l0 + ln], qps[:, :ln],
                                         AF.Identity, bias=bqc[:, hp:hp + 1])
                    kps = pps.tile([128, 512], F32, name="kps", tag="kps")
                    for k in range(KC):
                        nc.tensor.matmul(kps[:, :ln], wk[:, k, m], xt[:, k, l0:l0 + ln],
                                         start=(k == 0), stop=(k == KC - 1))
                    nc.scalar.copy(kt[:, hp, l0:l0 + ln], kps[:, :ln])

        # ---- attention ----
        with (
            tc.tile_pool(name="sps", bufs=2, space="PSUM") as sps,
            tc.tile_pool(name="cps", bufs=1, space="PSUM") as cps,
            tc.tile_pool(name="tps", bufs=2, space="PSUM") as tps,
            tc.tile_pool(name="wp", bufs=3) as wp,
        ):
            for hp in range(HP):
                hA, hB = 2 * hp, 2 * hp + 1
                for (l0, ln) in L_CHUNKS:
                    cA = cps.tile([65, 512], F32, name="cA", tag="cA")
                    cB = cps.tile([65, 512], F32, name="cB", tag="cB")
                    for si, (s0, ss) in enumerate(TILES):
                        stA = sps.tile([128, 512], F32, name="stA", tag="stA")
                        stB = sps.tile([128, 512], F32, name="stB", tag="stB")
                        nc.tensor.matmul(stA[:ss, :ln], kt[0:64, hp, s0:s0 + ss],
                                         qt[0:64, hp, l0:l0 + ln],
                                         start=True, stop=True, tile_position=(0, 0))
                        nc.tensor.matmul(stB[:ss, :ln], kt[64:128, hp, s0:s0 + ss],
                                         qt[64:128, hp, l0:l0 + ln],
                                         start=True, stop=True, tile_position=(64, 0))
                        eA = wp.tile([128, 512], DT, name="eA", tag="eA")
                        eB = wp.tile([128, 512], DT, name="eB", tag="eB")
                        nc.scalar.activation(eA[:ss, :ln], stA[:ss, :ln], AF.Exp,
                                             scale=0.125)
                        nc.scalar.activation(eB[:ss, :ln], stB[:ss, :ln], AF.Exp,
                                             scale=0.125)
                        nc.tensor.matmul(cA[:, :ln], vv[:ss, si, hA, :], eA[:ss, :ln],
                                         start=(si == 0), stop=(si == NS - 1))
                        nc.tensor.matmul(cB[:, :ln], vv[:ss, si, hB, :], eB[:ss, :ln],
                                         start=(si == 0), stop=(si == NS - 1))
                    ctA = wp.tile([65, 512], F32, name="ctA", tag="ctA")
                    ctB = wp.tile([65, 512], F32, name="ctB", tag="ctB")
                    nc.scalar.copy(ctA[:, :ln], cA[:, :ln])
                    nc.scalar.copy(ctB[:, :ln], cB[:, :ln])
                    for j in range(0, ln, 128):
                        lt = (l0 + j) // 128
                        w = min(128, ln - j)
                        for (h, ct) in ((hA, ctA), (hB, ctB)):
                            tr = tps.tile([128, 65], F32, name="tr", tag="tr")
                            nc.tensor.transpose(tr[:w, :], ct[:, j:j + w],
                                                ident[0:65, 0:65])
                            rc = wp.tile([128, 1], F32, name="rc", tag="rc")
                            nc.vector.reciprocal(rc[:w, :], tr[:w, 64:65])
                            nc.vector.tensor_scalar_mul(
                                ost[:w, lt, h * D:(h + 1) * D],
                                tr[:w, 0:D], rc[:w, :])

            for ti, (t0, tn) in enumerate(TILES):
                nc.sync.dma_start(out_d[t0:t0 + tn, :], ost[:tn, ti, :])


_NC_CACHE = {}


def _build():
    if "nc" in _NC_CACHE:
        return _NC_CACHE["nc"]
    nc = bacc.Bacc("TRN2", target_bir_lowering=False, debug=False)
    xt_d = nc.dram_tensor("xt", [HID, L], DT, kind="ExternalInput")
    wq_d = nc.dram_tensor("wqt", [HID, QD], DT, kind="ExternalInput")
    wk_d = nc.dram_tensor("wkt", [HID, QD], DT, kind="ExternalInput")
    wv_d = nc.dram_tensor("wvt", [HID, QD], DT, kind="ExternalInput")
    bq_d = nc.dram_tensor("bq", [QD, 1], F32, kind="ExternalInput")
    bv_d = nc.dram_tensor("bvb", [128, QD], F32, kind="ExternalInput")
    out_d = nc.dram_tensor("out", [L, QD], F32, kind="ExternalOutput")

    with tile.TileContext(nc) as tc:
        _body(nc, tc, xt_d.ap(), wq_d.ap(), wk_d.ap(), wv_d.ap(),
              bq_d.ap(), bv_d.ap(), out_d.ap())
    nc.compile()
    _NC_CACHE["nc"] = nc
    return nc


def make_in_maps(hidden_states, Wq, bq, Wk, bk, Wv, bv):
    in_maps = []
    for c in range(8):
        b, g = divmod(c, 2)
        gs = slice(g * QD, (g + 1) * QD)
        in_maps.append({
            "xt": np.ascontiguousarray(hidden_states[b].T).astype(NPDT),
            "wqt": np.ascontiguousarray(Wq[gs, :].T).astype(NPDT),
            "wkt": np.ascontiguousarray(Wk[gs, :].T).astype(NPDT),
            "wvt": np.ascontiguousarray(Wv[gs, :].T).astype(NPDT),
            "bq": bq[gs].reshape(QD, 1).astype(np.float32),
            "bvb": np.ascontiguousarray(
                np.broadcast_to(bv[gs], (128, QD))).astype(np.float32),
        })
    return in_maps


LAST_RESULTS = None


def kernel(hidden_states, Wq, bq, Wk, bk, Wv, bv):
    global LAST_RESULTS
    nc = _build()
    in_maps = make_in_maps(hidden_states, Wq, bq, Wk, bk, Wv, bv)
    res = bass_utils.run_bass_kernel_spmd(
        nc, in_maps, core_ids=list(range(8)),
        trace=bool(os.environ.get("KERNEL_TRACE")),
    )
    LAST_RESULTS = res
    out = np.empty((B, L, HID), np.float32)
    for c, om in enumerate(res.results):
        b, g = divmod(c, 2)
        out[b, :, g * QD:(g + 1) * QD] = om["out"]
    return out
